# revision 1
# baseline (speedup 1.0000x reference)
"""Sliding-window GQA attention (RoPE + tanh soft-cap) on 8 Trainium2 cores.

Sharding: core c = (b, g) with b = c // 4 (batch), g = c % 4 (head group).
Each core handles batch b, q-heads [4g, 4g+4), kv-heads [2g, 2g+2), computes a
partial output of shape [T, D]; the host sums the 4 partials per batch.

Device program per core (identical SPMD program, different input slices):
  A1: q^T = (q_w_scaled^T @ x^T), RoPE       -> DRAM scratch qTd [1024, 2048]
  A2: k^T = (k_w^T @ x^T), RoPE              -> SBUF resident [512, 2048]
  A3: v   = (x @ v_w)                        -> SBUF resident [2048, 512]
  B1: per 128-query block: logits = q^T.T @ k^T over the 1152-key band,
      tanh soft-cap (ACT), band mask add, softmax (max/exp/recip),
      PE-transpose probs, probs^T @ v -> DRAM scratch encd [1024, 2048]
  B2: out = enc^T.T @ o_w                    -> DRAM [2048, 3584]

Matmuls run as float32r (fp32 storage, reduced-precision full-rate PE mode).
"""

import numpy as np

B, T, D, N, KH, H = 2, 2048, 3584, 16, 8, 256
WINDOW = 1024
SOFT_CAP = 50.0
SCALAR = 0.0625
BASE = 10000.0
NEG = -1.0e6  # mask value in tanh-domain (exp(50*NEG) underflows to 0)

P = 128
NH = N // 4    # q heads per core = 4
NKH = KH // 4  # kv heads per core = 2
KT = D // P    # 28 contraction tiles
NA = NH * (H // P)   # 8 q^T row-tiles per core
KA = NKH * (H // P)  # 4 k^T row-tiles per core
TB = T // P    # 16 query blocks
BAND = 1152    # key band per query block (9 x 128)
TW = 256       # query-pair width for the transposed-attention phase

_PROG_CACHE = {}


def _build_program():
    import concourse.bacc as bacc
    import concourse.tile as tile
    import concourse.mybir as mybir
    from concourse.masks import make_identity

    F32 = mybir.dt.float32
    F32R = mybir.dt.float32r
    Tanh = mybir.ActivationFunctionType.Tanh
    Exp = mybir.ActivationFunctionType.Exp

    nc = bacc.Bacc("TRN2", target_bir_lowering=False, debug=False, num_devices=8)

    xT = nc.dram_tensor("xT", [D, T], F32, kind="ExternalInput")
    qw = nc.dram_tensor("qw", [D, NH * H], F32, kind="ExternalInput")
    kw = nc.dram_tensor("kw", [D, NKH * H], F32, kind="ExternalInput")
    vw = nc.dram_tensor("vw", [D, NKH * H], F32, kind="ExternalInput")
    ow = nc.dram_tensor("ow", [NH * H, D], F32, kind="ExternalInput")
    cosT = nc.dram_tensor("cosT", [P, T], F32, kind="ExternalInput")
    sinT = nc.dram_tensor("sinT", [P, T], F32, kind="ExternalInput")
    maskt = nc.dram_tensor("maskt", [4 * P, TW], F32, kind="ExternalInput")
    qTd = nc.dram_tensor("qTd", [NA * P, T], F32, kind="Internal")
    sums_d = nc.dram_tensor("sums_d", [NH, T], F32, kind="Internal")
    rcp_d = nc.dram_tensor("rcp_d", [NH, T], F32, kind="Internal")
    out_p = nc.dram_tensor("out_p", [T, D], F32, kind="ExternalOutput")

    xT_v = xT.ap().rearrange("(o p) t -> p o t", p=P)        # [128, 28, 2048]
    qw_v = qw.ap().rearrange("(o p) h -> p o h", p=P)        # [128, 28, 1024]
    kw_v = kw.ap().rearrange("(o p) h -> p o h", p=P)        # [128, 28, 512]
    vw_v = vw.ap().rearrange("(o p) h -> p o h", p=P)        # [128, 28, 512]
    ow_v = ow.ap().rearrange("(a p) d -> p a d", p=P)        # [128, 8, 3584]
    qTd_v = qTd.ap().rearrange("(a p) t -> p a t", p=P)      # [128, 8, 2048]
    out_v = out_p.ap()

    CH = 256             # t-chunk for projections
    NCH = T // CH        # 8

    def rope_pair(vec, dst_a, dst_b, src_a, src_b, cs, sn, tmp_pool, tag):
        # dst_a = src_a*cos - src_b*sin ; dst_b = src_b*cos + src_a*sin
        t1 = tmp_pool.tile([P, CH], F32, tag=tag)
        t2 = tmp_pool.tile([P, CH], F32, tag=tag)
        vec.tensor_mul(dst_a, src_a, cs)
        vec.tensor_mul(t1, src_b, sn)
        vec.tensor_sub(dst_a, dst_a.bitcast(F32), t1)
        vec.tensor_mul(dst_b, src_b, cs)
        vec.tensor_mul(t2, src_a, sn)
        vec.tensor_add(dst_b, dst_b.bitcast(F32), t2)

    with tile.TileContext(nc) as tc:
        with tc.tile_pool(name="p_tab", bufs=1) as p_tab:
            cos_sb = p_tab.tile([P, T], F32)
            sin_sb = p_tab.tile([P, T], F32)
            nc.sync.dma_start(cos_sb[:], cosT.ap())
            nc.sync.dma_start(sin_sb[:], sinT.ap())

            # ---------------- Phase A1: q^T -> qTd ----------------
            with tc.tile_pool(name="p_a1", bufs=1) as pa1, \
                 tc.tile_pool(name="p_a1x", bufs=8) as pa1x, \
                 tc.tile_pool(name="p_a1r", bufs=3) as pa1r, \
                 tc.tile_pool(name="p_a1o", bufs=2) as pa1o, \
                 tc.tile_pool(name="ps_a1", bufs=2, space="PSUM") as ps_a1:
                qw_sb = pa1.tile([P, KT, NH * H], F32R)      # 112 KB/part
                for ch in range(NCH):
                    tsl = slice(ch * CH, (ch + 1) * CH)
                    ps = ps_a1.tile([P, NA, CH], F32, tag="qps")
                    for k in range(KT):
                        if ch == 0:
                            nc.sync.dma_start(qw_sb[:, k], qw_v[:, k].bitcast(F32R))
                        xt = pa1x.tile([P, CH], F32R, tag="xt")
                        nc.sync.dma_start(xt[:], xT_v[:, k, tsl].bitcast(F32R))
                        for j in range(NA):
                            # PSUM start=True clears the whole bank; banks hold
                            # two j-groups, so only the even j issues the clear.
                            nc.tensor.matmul(
                                ps[:, j], qw_sb[:, k, j * P:(j + 1) * P], xt[:],
                                start=(k == 0 and j % 2 == 0),
                                stop=(k == KT - 1), skip_group_check=True)
                    qto = pa1o.tile([P, NA, CH], F32R, tag="qto")
                    cs, sn = cos_sb[:, tsl], sin_sb[:, tsl]
                    for pr in range(NA // 2):
                        rope_pair(nc.vector, qto[:, 2 * pr], qto[:, 2 * pr + 1],
                                  ps[:, 2 * pr], ps[:, 2 * pr + 1], cs, sn,
                                  pa1r, "rtmp")
                    nc.sync.dma_start(qTd_v[:, :, tsl].bitcast(F32R), qto[:])

            with tc.tile_pool(name="p_kv", bufs=1) as p_kv:
                kT_sb = p_kv.tile([P, KA, T], F32R)          # 32 KB/part
                v_sb = p_kv.tile([P, TB, NKH * H], F32R)     # 32 KB/part

                # ---------------- Phase A2+A3: k^T and v, one xT pass ----------
                with tc.tile_pool(name="p_a2", bufs=1) as pa2, \
                     tc.tile_pool(name="p_a2x", bufs=8) as pa2x, \
                     tc.tile_pool(name="p_a2r", bufs=3) as pa2r, \
                     tc.tile_pool(name="ps_a2", bufs=2, space="PSUM") as ps_a2, \
                     tc.tile_pool(name="ps_a3", bufs=2, space="PSUM") as ps_a3:
                    kw_sb = pa2.tile([P, KT, NKH * H], F32R)  # 56 KB/part
                    vw_sb = pa2.tile([P, KT, NKH * H], F32R)  # 56 KB/part
                    for ch in range(NCH):
                        tsl = slice(ch * CH, (ch + 1) * CH)
                        ps = ps_a2.tile([P, KA, CH], F32, tag="kps")
                        psv = ps_a3.tile([P, CH // P, NKH * H], F32, tag="vps")
                        for k in range(KT):
                            if ch == 0:
                                nc.sync.dma_start(kw_sb[:, k],
                                                  kw_v[:, k].bitcast(F32R))
                                nc.sync.dma_start(vw_sb[:, k],
                                                  vw_v[:, k].bitcast(F32R))
                            xt = pa2x.tile([P, CH], F32R, tag="xt")
                            nc.sync.dma_start(xt[:], xT_v[:, k, tsl].bitcast(F32R))
                            for j in range(KA):
                                nc.tensor.matmul(
                                    ps[:, j], kw_sb[:, k, j * P:(j + 1) * P], xt[:],
                                    start=(k == 0 and j % 2 == 0),
                                    stop=(k == KT - 1), skip_group_check=True)
                            for st in range(CH // P):
                                nc.tensor.matmul(
                                    psv[:, st], xt[:, st * P:(st + 1) * P],
                                    vw_sb[:, k], start=(k == 0), stop=(k == KT - 1))
                        cs, sn = cos_sb[:, tsl], sin_sb[:, tsl]
                        for pr in range(KA // 2):
                            rope_pair(nc.vector, kT_sb[:, 2 * pr, tsl],
                                      kT_sb[:, 2 * pr + 1, tsl],
                                      ps[:, 2 * pr], ps[:, 2 * pr + 1], cs, sn,
                                      pa2r, "rtmp")
                        for st in range(CH // P):
                            nc.vector.tensor_copy(
                                v_sb[:, ch * (CH // P) + st, :], psv[:, st])

                # ---------------- Phase B1: attention -> enc_sb ----------------
                # Transposed orientation: logits^T [s, t] via kT-stationary
                # matmuls over 256-query pairs; softmax without row reduction
                # (constant exp shift); per-t sums via ones-matmul; the 1/sum
                # normalization is applied in B2. No PE transposes needed, and
                # enc^T accumulates directly in SBUF (no DRAM roundtrip).
                import concourse.bass as bass_mod
                with tc.tile_pool(name="p_enc", bufs=1) as p_enc:
                    enc_sb = p_enc.tile([P, NA, T], F32R)    # 64 KB/part
                    with tc.tile_pool(name="p_b1", bufs=1) as pb1, \
                         tc.tile_pool(name="p_b1s", bufs=2) as pb1s, \
                         tc.tile_pool(name="p_b1e", bufs=3) as pb1e, \
                         tc.tile_pool(name="ps_lg", bufs=2, space="PSUM") as ps_lg, \
                         tc.tile_pool(name="ps_sm", bufs=2, space="PSUM") as ps_sm, \
                         tc.tile_pool(name="ps_en", bufs=2, space="PSUM") as ps_en:
                        mk_sb = pb1.tile([P, 4, TW], F32)
                        nc.sync.dma_start(mk_sb[:], maskt.ap().rearrange(
                            "(m p) c -> p m c", p=P))
                        bias_c = pb1.tile([P, 1], F32)
                        nc.vector.memset(bias_c[:], -10.0)
                        ones_sb = pb1.tile([P, 1], F32R)
                        nc.vector.memset(ones_sb[:].bitcast(F32), 1.0)
                        MKJ = {0: 0, 1: 1, 8: 2, 9: 3}

                        for pr in range(T // TW):
                            t0p = pr * TW
                            qb = pb1s.tile([P, NA, TW], F32R, tag="qb")
                            nc.sync.dma_start(
                                qb[:], qTd_v[:, :, t0p:t0p + TW].bitcast(F32R))
                            js = max(0, 8 - 2 * pr)
                            for kh in range(NKH):
                                for nl in range(2):
                                    n = kh * 2 + nl
                                    exps = pb1e.tile([P, 10, TW], F32R, tag="expT")
                                    smp = ps_sm.tile([P, TW], F32, tag="smp")
                                    encp = ps_en.tile([P, 2, TW], F32, tag="en")
                                    jgroups = []
                                    j = js
                                    while j < 10:
                                        w = min(4, 10 - j)
                                        jgroups.append((j, w))
                                        j += w
                                    for gi, (j0, w) in enumerate(jgroups):
                                        lgT = ps_lg.tile([P, 4, TW], F32, tag="lgt")
                                        for dj in range(w):
                                            j = j0 + dj
                                            s0 = (2 * pr - 8 + j) * P
                                            for hh in range(2):
                                                nc.tensor.matmul(
                                                    lgT[:, dj],
                                                    kT_sb[:, kh * 2 + hh,
                                                          s0:s0 + P],
                                                    qb[:, n * 2 + hh],
                                                    start=(hh == 0 and
                                                           dj % 2 == 0),
                                                    stop=(hh == 1),
                                                    skip_group_check=True)
                                        tT = pb1s.tile([P, 4, TW], F32, tag="tT")
                                        nc.scalar.activation(
                                            tT[:, :w], lgT[:, :w], Tanh,
                                            scale=1.0 / SOFT_CAP)
                                        for dj in range(w):
                                            j = j0 + dj
                                            if j in MKJ:
                                                nc.vector.tensor_add(
                                                    tT[:, dj], tT[:, dj],
                                                    mk_sb[:, MKJ[j]])
                                        nc.scalar.activation(
                                            exps[:, j0:j0 + w], tT[:, :w],
                                            bias=bias_c[:], func=Exp,
                                            scale=SOFT_CAP)
                                        for dj in range(w):
                                            nc.tensor.matmul(
                                                smp[0:1, :], ones_sb[:],
                                                exps[:, j0 + dj],
                                                start=(gi == 0 and dj == 0),
                                                stop=(j0 + dj == 9),
                                                skip_group_check=True)
                                        for dj in range(w):
                                            j = j0 + dj
                                            stg = 2 * pr - 8 + j
                                            for hh in range(2):
                                                nc.tensor.matmul(
                                                    encp[:, hh],
                                                    v_sb[:, stg,
                                                         kh * H + hh * P:
                                                         kh * H + (hh + 1) * P],
                                                    exps[:, j],
                                                    start=(gi == 0 and dj == 0
                                                           and hh == 0),
                                                    stop=(j == 9),
                                                    skip_group_check=True)
                                    srow = pb1s.tile([1, TW], F32, tag="srow")
                                    nc.vector.tensor_copy(srow[:], smp[0:1, :])
                                    nc.sync.dma_start(
                                        sums_d.ap()[n:n + 1, t0p:t0p + TW],
                                        srow[:])
                                    # enc row-tile order: a = kh*4 + nl*2 + hh
                                    for hh in range(2):
                                        a = 4 * kh + 2 * nl + hh
                                        nc.vector.tensor_copy(
                                            enc_sb[:, a, t0p:t0p + TW],
                                            encp[:, hh])

                    # ---------------- Phase B2: output projection ----------------
                    with tc.tile_pool(name="p_b2", bufs=2) as pb2, \
                         tc.tile_pool(name="p_b2n", bufs=2) as pb2n, \
                         tc.tile_pool(name="p_b2o", bufs=3) as pb2o, \
                         tc.tile_pool(name="ps_b2", bufs=2, space="PSUM") as ps_b2:
                        for n in range(NH):
                            rbc = pb2n.tile([P, T], F32, tag="rbc")
                            row = sums_d.ap()[n:n + 1, :]
                            bcast = bass_mod.AP(
                                tensor=row.tensor, offset=row.offset,
                                ap=[[0, P]] + [list(d) for d in row.ap[1:]])
                            nc.sync.dma_start(rbc[:], bcast)
                            nc.vector.reciprocal(rbc[:], rbc[:])
                            kh, nl = divmod(n, 2)
                            for hh in range(2):
                                a = kh * 4 + nl * 2 + hh
                                nc.vector.tensor_mul(
                                    enc_sb[:, a], enc_sb[:, a].bitcast(F32),
                                    rbc[:])
                        for dch in range(D // 512):
                            dsl = slice(dch * 512, (dch + 1) * 512)
                            ow_sb = pb2.tile([P, NA, 512], F32R, tag="ow")
                            nc.sync.dma_start(ow_sb[:],
                                              ow_v[:, :, dsl].bitcast(F32R))
                            for tb in range(TB):
                                t0 = tb * P
                                po = ps_b2.tile([P, 512], F32, tag="po")
                                for a in range(NA):
                                    nc.tensor.matmul(
                                        po[:], enc_sb[:, a, t0:t0 + P],
                                        ow_sb[:, a],
                                        start=(a == 0), stop=(a == NA - 1))
                                ob = pb2o.tile([P, 512], F32, tag="ob")
                                nc.scalar.copy(ob[:], po[:])
                                nc.sync.dma_start(out_v[t0:t0 + P, dsl], ob[:])

    nc.compile()
    return nc


def _get_program():
    if "nc" not in _PROG_CACHE:
        _PROG_CACHE["nc"] = _build_program()
    return _PROG_CACHE["nc"]


def _host_inputs(x, segment_pos, q_w, kv_w, o_w):
    """Build the 8 per-core input dicts."""
    xTs = [np.ascontiguousarray(x[b].T) for b in range(B)]
    tabs = []
    for b in range(B):
        pos = segment_pos[b].astype(np.float64)
        inv_ts = BASE ** (-2.0 * np.arange(H // 2, dtype=np.float64) / H)
        ang = inv_ts[:, None] * pos[None, :]          # [128, T]
        tabs.append((np.cos(ang).astype(np.float32),
                     np.sin(ang).astype(np.float32)))

    # transposed-band masks for s-tiles j in {0,1,8,9}:
    # valid  <=>  128j + i - 1024 <= c <= 128j + i - 1   (i: s within tile,
    # c: query offset within the 256-wide pair)
    i = np.arange(P)[:, None]
    c = np.arange(TW)[None, :]
    tiles = []
    for j in (0, 1, 8, 9):
        valid = (c >= P * j + i - WINDOW) & (c <= P * j + i - 1)
        tiles.append(np.where(valid, np.float32(0.0), np.float32(NEG)))
    maskt = np.concatenate(tiles, axis=0)

    in_maps = []
    for core in range(8):
        b, g = core // 4, core % 4
        qws = np.ascontiguousarray(
            q_w[4 * g:4 * g + 4].transpose(1, 0, 2).reshape(D, NH * H)
        ) * np.float32(SCALAR)
        kws = np.ascontiguousarray(
            kv_w[0, 2 * g:2 * g + 2].transpose(1, 0, 2).reshape(D, NKH * H))
        vws = np.ascontiguousarray(
            kv_w[1, 2 * g:2 * g + 2].transpose(1, 0, 2).reshape(D, NKH * H))
        # row-tile order a = kh*4 + nl*2 + hh, matching encd's B1 write layout
        ow_tiles = []
        for a in range(NA):
            kh, r = divmod(a, 4)
            nl, hh = divmod(r, 2)
            ow_tiles.append(o_w[4 * g + 2 * kh + nl, hh * P:(hh + 1) * P, :])
        ows = np.ascontiguousarray(np.concatenate(ow_tiles, axis=0))
        in_maps.append({
            "xT": xTs[b], "qw": qws, "kw": kws, "vw": vws, "ow": ows,
            "cosT": tabs[b][0], "sinT": tabs[b][1], "maskt": maskt,
        })
    return in_maps


def kernel(x, segment_pos, attn_mask, q_w, kv_w, o_w):
    from concourse import bass_utils

    x = np.asarray(x, dtype=np.float32)
    q_w = np.asarray(q_w, dtype=np.float32)
    kv_w = np.asarray(kv_w, dtype=np.float32)
    o_w = np.asarray(o_w, dtype=np.float32)
    segment_pos = np.asarray(segment_pos)

    nc = _get_program()
    in_maps = _host_inputs(x, segment_pos, q_w, kv_w, o_w)
    res = bass_utils.run_bass_kernel_spmd(nc, in_maps, core_ids=list(range(8)))
    out = np.zeros((B, T, D), dtype=np.float32)
    for core in range(8):
        out[core // 4] += res.results[core]["out_p"]
    return out



# revision 11
# speedup vs baseline: 153.7080x; 153.7080x over previous
"""Sliding-window GQA attention (RoPE + tanh soft-cap) on 8 Trainium2 cores.

Sharding: core c = (b, g) with b = c // 4 (batch), g = c % 4 (head group).
Each core handles batch b, q-heads [4g, 4g+4), kv-heads [2g, 2g+2), computes a
partial output of shape [T, D]; the host sums the 4 partials per batch.

Device program per core (identical SPMD program, different input slices),
all-bf16 datapath (PSUM accumulation in f32):
  A1: q^T = (q_w_scaled^T @ x^T), RoPE      -> SBUF resident qT [8x128, 2048]
  A2: k^T = (k_w^T @ x^T), RoPE             -> SBUF resident [4x128, 2048]
  A3: v   = (x @ v_w)                       -> SBUF resident [2048, 512]
  B1: per 256-query pair: transposed logits = kT-stationary matmuls over the
      10-tile key band, tanh soft-cap (ACT), band mask add, exp with constant
      shift, per-t sums via ones-matmul, 1/sum broadcast via PE outer
      product, enc^T = (v-stationary @ probs^T) * (1/sum) -> SBUF bf16
  B2: out = enc^T.T @ o_w                   -> DRAM [2048, 3584] bf16
"""

import numpy as np

B, T, D, N, KH, H = 2, 2048, 3584, 16, 8, 256
WINDOW = 1024
SOFT_CAP = 50.0
SCALAR = 0.0625
BASE = 10000.0
NEG = -1.0e6  # mask value in tanh-domain (exp(50*NEG) underflows to 0)

P = 128
NH = N // 4    # q heads per core = 4
NKH = KH // 4  # kv heads per core = 2
KT = D // P    # 28 contraction tiles
NA = NH * (H // P)   # 8 q^T row-tiles per core
KA = NKH * (H // P)  # 4 k^T row-tiles per core
TB = T // P    # 16 query blocks
TW = 256       # query-pair width for the transposed-attention phase

_PROG_CACHE = {}


def _build_program(reps=1):
    """Build the device program. reps>1 repeats the whole computation in
    sequence on-device (identical math each iteration, same outputs) — used
    by test.py to measure per-iteration HW execution time without the
    dispatch-overhead floor. The graded kernel() path uses reps=1."""
    import concourse.bacc as bacc
    import concourse.tile as tile
    import concourse.mybir as mybir

    F32 = mybir.dt.float32
    BF = mybir.dt.bfloat16
    Tanh = mybir.ActivationFunctionType.Tanh
    Exp = mybir.ActivationFunctionType.Exp

    nc = bacc.Bacc("TRN2", target_bir_lowering=False, debug=False, num_devices=8)

    xT = nc.dram_tensor("xT", [D, T], BF, kind="ExternalInput")
    qw = nc.dram_tensor("qw", [D, NH * H], BF, kind="ExternalInput")
    kw = nc.dram_tensor("kw", [D, NKH * H], BF, kind="ExternalInput")
    vw = nc.dram_tensor("vw", [D, NKH * H], BF, kind="ExternalInput")
    ow = nc.dram_tensor("ow", [NH * H, D], BF, kind="ExternalInput")
    cosT = nc.dram_tensor("cosT", [P, T], F32, kind="ExternalInput")
    sinT = nc.dram_tensor("sinT", [P, T], F32, kind="ExternalInput")
    maskt = nc.dram_tensor("maskt", [4 * P, TW], F32, kind="ExternalInput")
    out_p = nc.dram_tensor("out_p", [T, D], BF, kind="ExternalOutput")

    xT_v = xT.ap().rearrange("(o p) t -> p o t", p=P)        # [128, 28, 2048]
    qw_v = qw.ap().rearrange("(o p) h -> p o h", p=P)        # [128, 28, 1024]
    kw_v = kw.ap().rearrange("(o p) h -> p o h", p=P)        # [128, 28, 512]
    vw_v = vw.ap().rearrange("(o p) h -> p o h", p=P)        # [128, 28, 512]
    ow_v = ow.ap().rearrange("(a p) d -> p a d", p=P)        # [128, 8, 3584]
    out_v = out_p.ap()

    CH = 256             # t-chunk for projections
    NCH = T // CH        # 8

    def rope_pair(vec, dst_a, dst_b, src_a, src_b, cs, sn, tmp_pool, tag):
        # dst_a = src_a*cos - src_b*sin ; dst_b = src_b*cos + src_a*sin
        # (f32 temporaries; dst tiles are bf16 slices of the resident buffers)
        t1 = tmp_pool.tile([P, CH], F32, tag=tag)
        t2 = tmp_pool.tile([P, CH], F32, tag=tag)
        t3 = tmp_pool.tile([P, CH], F32, tag=tag)
        t4 = tmp_pool.tile([P, CH], F32, tag=tag)
        vec.tensor_mul(t1, src_a, cs)
        vec.tensor_mul(t2, src_b, sn)
        vec.tensor_sub(dst_a, t1, t2)
        vec.tensor_mul(t3, src_b, cs)
        vec.tensor_mul(t4, src_a, sn)
        vec.tensor_add(dst_b, t3, t4)

    with tile.TileContext(nc) as tc, \
         nc.allow_low_precision(reason="bf16 datapath; f32 PSUM accumulate"):
      for _rep in range(reps):
        with tc.tile_pool(name="p_tab", bufs=1) as p_tab, \
             tc.tile_pool(name="p_res", bufs=1) as p_res:
            cos_sb = p_tab.tile([P, T], F32)
            sin_sb = p_tab.tile([P, T], F32)

            qT_sb = p_res.tile([P, NA, T], BF)       # 32 KB/part
            kT_sb = p_res.tile([P, KA, T], BF)       # 16 KB/part
            v_sb = p_res.tile([P, TB, NKH * H], BF)  # 16 KB/part
            enc_sb = p_res.tile([P, NA, T], BF)      # 32 KB/part

            # ---------------- Phase A1: q^T -> qT_sb ----------------
            with tc.tile_pool(name="p_a1", bufs=1) as pa1, \
                 tc.tile_pool(name="p_a1x", bufs=8) as pa1x, \
                 tc.tile_pool(name="p_a1r", bufs=3) as pa1r, \
                 tc.tile_pool(name="ps_a1", bufs=2, space="PSUM") as ps_a1:
                qw_sb = pa1.tile([P, KT, NH * H], BF)      # 56 KB/part
                for ch in range(NCH):
                    tsl = slice(ch * CH, (ch + 1) * CH)
                    ps = ps_a1.tile([P, NA, CH], F32, tag="qps")
                    for k in range(KT):
                        if ch == 0:
                            nc.sync.dma_start(qw_sb[:, k], qw_v[:, k])
                        xt = pa1x.tile([P, CH], BF, tag="xt")
                        nc.sync.dma_start(xt[:], xT_v[:, k, tsl])
                        for j in range(NA):
                            # PSUM start=True clears the whole bank; banks hold
                            # two j-groups, so only the even j issues the clear.
                            nc.tensor.matmul(
                                ps[:, j], qw_sb[:, k, j * P:(j + 1) * P], xt[:],
                                start=(k == 0 and j % 2 == 0),
                                stop=(k == KT - 1), skip_group_check=True)
                    if ch == 0:
                        # issued here (not at kernel start) so the first
                        # weight/x loads own the DMA queues at t=0
                        nc.sync.dma_start(cos_sb[:], cosT.ap())
                        nc.sync.dma_start(sin_sb[:], sinT.ap())
                    cs, sn = cos_sb[:, tsl], sin_sb[:, tsl]
                    for pr in range(NA // 2):
                        rope_pair(nc.vector, qT_sb[:, 2 * pr, tsl],
                                  qT_sb[:, 2 * pr + 1, tsl],
                                  ps[:, 2 * pr], ps[:, 2 * pr + 1], cs, sn,
                                  pa1r, "rtmp")

            # ---------------- Phase A2+A3: k^T and v, one xT pass ----------
            with tc.tile_pool(name="p_a2", bufs=1) as pa2, \
                 tc.tile_pool(name="p_a2x", bufs=8) as pa2x, \
                 tc.tile_pool(name="p_a2r", bufs=3) as pa2r, \
                 tc.tile_pool(name="ps_a2", bufs=2, space="PSUM") as ps_a2, \
                 tc.tile_pool(name="ps_a3", bufs=2, space="PSUM") as ps_a3:
                kw_sb = pa2.tile([P, KT, NKH * H], BF)  # 28 KB/part
                vw_sb = pa2.tile([P, KT, NKH * H], BF)  # 28 KB/part
                for ch in range(NCH):
                    tsl = slice(ch * CH, (ch + 1) * CH)
                    ps = ps_a2.tile([P, KA, CH], F32, tag="kps")
                    psv = ps_a3.tile([P, CH // P, NKH * H], F32, tag="vps")
                    for k in range(KT):
                        if ch == 0:
                            nc.sync.dma_start(kw_sb[:, k], kw_v[:, k])
                            nc.sync.dma_start(vw_sb[:, k], vw_v[:, k])
                        xt = pa2x.tile([P, CH], BF, tag="xt")
                        nc.sync.dma_start(xt[:], xT_v[:, k, tsl])
                        for j in range(KA):
                            nc.tensor.matmul(
                                ps[:, j], kw_sb[:, k, j * P:(j + 1) * P], xt[:],
                                start=(k == 0 and j % 2 == 0),
                                stop=(k == KT - 1), skip_group_check=True)
                        for st in range(CH // P):
                            nc.tensor.matmul(
                                psv[:, st], xt[:, st * P:(st + 1) * P],
                                vw_sb[:, k], start=(k == 0), stop=(k == KT - 1))
                    cs, sn = cos_sb[:, tsl], sin_sb[:, tsl]
                    for pr in range(KA // 2):
                        rope_pair(nc.vector, kT_sb[:, 2 * pr, tsl],
                                  kT_sb[:, 2 * pr + 1, tsl],
                                  ps[:, 2 * pr], ps[:, 2 * pr + 1], cs, sn,
                                  pa2r, "rtmp")
                    for st in range(CH // P):
                        nc.vector.tensor_copy(
                            v_sb[:, ch * (CH // P) + st, :], psv[:, st])

            # ---------------- Phase B1: attention -> enc_sb ----------------
            # Transposed orientation: logits^T [s, t] via kT-stationary
            # matmuls over 256-query pairs; softmax without row reduction
            # (constant exp shift); per-t sums via ones-matmul; 1/sum is
            # broadcast across partitions with a K=1 PE outer product and
            # folded into the PSUM->SBUF move of enc (no B2 prologue).
            with tc.tile_pool(name="p_b1", bufs=1) as pb1, \
                 tc.tile_pool(name="p_b1s", bufs=2) as pb1s, \
                 tc.tile_pool(name="p_b1r", bufs=3) as pb1r, \
                 tc.tile_pool(name="p_b1e", bufs=3) as pb1e, \
                 tc.tile_pool(name="p_b2", bufs=1) as pb2, \
                 tc.tile_pool(name="p_b2o", bufs=3) as pb2o, \
                 tc.tile_pool(name="ps_lg", bufs=2, space="PSUM") as ps_lg, \
                 tc.tile_pool(name="ps_sm", bufs=1, space="PSUM") as ps_sm, \
                 tc.tile_pool(name="ps_en", bufs=1, space="PSUM") as ps_en, \
                 tc.tile_pool(name="ps_b2", bufs=2, space="PSUM") as ps_b2:
                # B2's o_w stays fully resident so output-projection tiles can
                # interleave with B1 as soon as their enc columns finalize.
                ow_sb = pb2.tile([P, NA, D], BF)   # 56 KB/part
                nc.sync.dma_start(ow_sb[:], ow_v[:])
                mk_sb = pb1.tile([P, 4, TW], F32)
                nc.sync.dma_start(mk_sb[:], maskt.ap().rearrange(
                    "(m p) c -> p m c", p=P))
                bias_c = pb1.tile([P, 1], F32)
                nc.vector.memset(bias_c[:], -10.0)
                ones_sb = pb1.tile([P, 1], BF)
                nc.vector.memset(ones_sb[:], 1.0)
                ones_row = pb1.tile([1, P], BF)
                nc.vector.memset(ones_row[:], 1.0)
                MKJ = {0: 0, 1: 1, 8: 2, 9: 3}

                for pr in range(T // TW):
                    t0p = pr * TW
                    js = max(0, 8 - 2 * pr)
                    for kh in range(NKH):
                        for nl in range(2):
                            n = kh * 2 + nl
                            exps = pb1e.tile([P, 10, TW], BF, tag="expT")
                            smp = ps_sm.tile([P, TW], F32, tag="smp")
                            encp = ps_en.tile([P, 2, TW], F32, tag="en")
                            jgroups = []
                            j = js
                            while j < 10:
                                w = min(4, 10 - j)
                                jgroups.append((j, w))
                                j += w
                            for gi, (j0, w) in enumerate(jgroups):
                                lgT = ps_lg.tile([P, 4, TW], F32, tag="lgt")
                                for dj in range(w):
                                    j = j0 + dj
                                    s0 = (2 * pr - 8 + j) * P
                                    for hh in range(2):
                                        nc.tensor.matmul(
                                            lgT[:, dj],
                                            kT_sb[:, kh * 2 + hh, s0:s0 + P],
                                            qT_sb[:, n * 2 + hh,
                                                  t0p:t0p + TW],
                                            start=(hh == 0 and dj % 2 == 0),
                                            stop=(hh == 1),
                                            skip_group_check=True)
                                tT = pb1s.tile([P, 4, TW], F32, tag="tT")
                                nc.scalar.activation(
                                    tT[:, :w], lgT[:, :w], Tanh,
                                    scale=1.0 / SOFT_CAP)
                                for dj in range(w):
                                    j = j0 + dj
                                    if j in MKJ:
                                        nc.vector.tensor_add(
                                            tT[:, dj], tT[:, dj],
                                            mk_sb[:, MKJ[j]])
                                nc.scalar.activation(
                                    exps[:, j0:j0 + w], tT[:, :w],
                                    bias=bias_c[:], func=Exp,
                                    scale=SOFT_CAP)
                                for dj in range(w):
                                    nc.tensor.matmul(
                                        smp[0:1, :], ones_sb[:],
                                        exps[:, j0 + dj],
                                        start=(gi == 0 and dj == 0),
                                        stop=(j0 + dj == 9),
                                        skip_group_check=True)
                                for dj in range(w):
                                    j = j0 + dj
                                    stg = 2 * pr - 8 + j
                                    for hh in range(2):
                                        nc.tensor.matmul(
                                            encp[:, hh],
                                            v_sb[:, stg,
                                                 kh * H + hh * P:
                                                 kh * H + (hh + 1) * P],
                                            exps[:, j],
                                            start=(gi == 0 and dj == 0
                                                   and hh == 0),
                                            stop=(j == 9),
                                            skip_group_check=True)
                            # 1/sum, broadcast to all partitions via PE
                            srow = pb1s.tile([1, TW], BF, tag="srow")
                            nc.vector.reciprocal(srow[:], smp[0:1, :])
                            psb = ps_lg.tile([P, 4, TW], F32, tag="lgt")
                            nc.tensor.matmul(psb[:, 0], ones_row[:], srow[:],
                                             start=True, stop=True,
                                             skip_group_check=True)
                            rbc = pb1r.tile([P, TW], F32, tag="rbc")
                            nc.vector.tensor_copy(rbc[:], psb[:, 0])
                            # enc row-tile order: a = kh*4 + nl*2 + hh
                            for hh in range(2):
                                a = 4 * kh + 2 * nl + hh
                                nc.vector.tensor_mul(
                                    enc_sb[:, a, t0p:t0p + TW],
                                    encp[:, hh], rbc[:])

                    # ---- B2 output projection, lagged one pr ----
                    # Emit output-projection tiles for the PREVIOUS pr's two
                    # t-blocks (their enc DVE writes have had a full pr to
                    # drain), so PE interleaves B2 without stalling on DVE.
                    b2_prs = [pr - 1] if pr < T // TW - 1 else [pr - 1, pr]
                    for bpr in b2_prs:
                        if bpr < 0:
                            continue
                        for tb in (2 * bpr, 2 * bpr + 1):
                            t0 = tb * P
                            for dch in range(D // 512):
                                dsl = slice(dch * 512, (dch + 1) * 512)
                                po = ps_b2.tile([P, 512], F32, tag="po")
                                for a in range(NA):
                                    nc.tensor.matmul(
                                        po[:], enc_sb[:, a, t0:t0 + P],
                                        ow_sb[:, a, dsl],
                                        start=(a == 0), stop=(a == NA - 1))
                                ob = pb2o.tile([P, 512], BF, tag="ob")
                                nc.scalar.copy(ob[:], po[:])
                                nc.sync.dma_start(out_v[t0:t0 + P, dsl], ob[:])

    nc.compile()
    return nc


def _get_program():
    if "nc" not in _PROG_CACHE:
        _PROG_CACHE["nc"] = _build_program()
    return _PROG_CACHE["nc"]


def _bf16(a):
    import ml_dtypes
    return np.asarray(a, dtype=np.float32).astype(ml_dtypes.bfloat16)


def _host_inputs(x, segment_pos, q_w, kv_w, o_w):
    """Build the 8 per-core input dicts."""
    xTs = [np.ascontiguousarray(_bf16(x[b].T)) for b in range(B)]
    tabs = []
    for b in range(B):
        pos = segment_pos[b].astype(np.float64)
        inv_ts = BASE ** (-2.0 * np.arange(H // 2, dtype=np.float64) / H)
        ang = inv_ts[:, None] * pos[None, :]          # [128, T]
        tabs.append((np.cos(ang).astype(np.float32),
                     np.sin(ang).astype(np.float32)))

    # transposed-band masks for s-tiles j in {0,1,8,9}:
    # valid  <=>  128j + i - 1024 <= c <= 128j + i - 1   (i: s within tile,
    # c: query offset within the 256-wide pair)
    i = np.arange(P)[:, None]
    c = np.arange(TW)[None, :]
    tiles = []
    for j in (0, 1, 8, 9):
        valid = (c >= P * j + i - WINDOW) & (c <= P * j + i - 1)
        tiles.append(np.where(valid, np.float32(0.0), np.float32(NEG)))
    maskt = np.concatenate(tiles, axis=0)

    in_maps = []
    for core in range(8):
        b, g = core // 4, core % 4
        qws = _bf16(np.ascontiguousarray(
            q_w[4 * g:4 * g + 4].transpose(1, 0, 2).reshape(D, NH * H)
        ) * np.float32(SCALAR))
        kws = _bf16(np.ascontiguousarray(
            kv_w[0, 2 * g:2 * g + 2].transpose(1, 0, 2).reshape(D, NKH * H)))
        vws = _bf16(np.ascontiguousarray(
            kv_w[1, 2 * g:2 * g + 2].transpose(1, 0, 2).reshape(D, NKH * H)))
        # row-tile order a = kh*4 + nl*2 + hh, matching enc_sb's B1 layout
        ow_tiles = []
        for a in range(NA):
            kh, r = divmod(a, 4)
            nl, hh = divmod(r, 2)
            ow_tiles.append(o_w[4 * g + 2 * kh + nl, hh * P:(hh + 1) * P, :])
        ows = _bf16(np.ascontiguousarray(np.concatenate(ow_tiles, axis=0)))
        in_maps.append({
            "xT": xTs[b], "qw": qws, "kw": kws, "vw": vws, "ow": ows,
            "cosT": tabs[b][0], "sinT": tabs[b][1], "maskt": maskt,
        })
    return in_maps


def kernel(x, segment_pos, attn_mask, q_w, kv_w, o_w):
    from concourse import bass_utils

    x = np.asarray(x, dtype=np.float32)
    q_w = np.asarray(q_w, dtype=np.float32)
    kv_w = np.asarray(kv_w, dtype=np.float32)
    o_w = np.asarray(o_w, dtype=np.float32)
    segment_pos = np.asarray(segment_pos)

    nc = _get_program()
    in_maps = _host_inputs(x, segment_pos, q_w, kv_w, o_w)
    res = bass_utils.run_bass_kernel_spmd(nc, in_maps, core_ids=list(range(8)))
    out = np.zeros((B, T, D), dtype=np.float32)
    for core in range(8):
        out[core // 4] += np.asarray(res.results[core]["out_p"],
                                     dtype=np.float32)
    return out
